# revision 36
# baseline (speedup 1.0000x reference)
"""Trainium2 Bass kernel for nn_AudioEvent: oscillator bank + FFT-filtered noise synth.

Sharding: data-parallel over batch (B=8) -> one batch element per NeuronCore.

Design (chunk-basis formulation):
  - phase(n) within each 512-sample chunk = frac(fp64 prefix) + 4-knot coef @
    fixed cumulative basis -> ONE fp16 hi/lo matmul (K=15) per (block, chunk);
    frequency values match the reference's fp32 rounding exactly.
  - range reduction: yt = fp16(phase + 1024) rounds to the nearest integer
    (fp16 ulp is exactly 1 in [1024, 2048)); -dt = (yt - 1024) - phase via one
    scalar_tensor_tensor; the sign flip is folded into negated env weights.
  - sin via ACT with scale just under 2*pi (arg stays inside [-pi, pi]).
  - envelope multiply folded into PE: per z-slice, left/right-knot env-valued
    selection matmuls (Glo/Ghi) sum the 33 oscillators per event; posc =
    w0*Glo + w1*Ghi with a fixed sawtooth weight profile (DVE).
  - results accumulate in z-major [16z+e, 256j] PSUM layout; the overall-env
    mix interp lands in a parallel bank; final mix with noise on GPSIMD.
  - noise: host pre-windowed+transposed frames; rDFT as fp16 matmuls keeping
    only bins 0..127 (gaussian filter ~0 above 0.5 Nyquist for this f0/std
    range); inverse rDFT fused with the overlap-add via a frame-shifted
    stationary operand, output directly in z-major layout.
  - emission is software-pipelined: selection matmuls lag one iteration
    behind the phase/sin front so the PE never waits on fresh sins.
"""
import os
import numpy as np

B = 8
NE = 16
NH = 32
SEQ = 64
N = 16384
WS = 512
NYQ = 11025.0
MIN_F0 = np.float32(20.0 / NYQ)
MAX_F0 = np.float32(800.0 / NYQ)
F0_DIFF = np.float32(MAX_F0 - MIN_F0)
NROW = NE * 33          # 528 osc rows
NBLK = 5                # 640 padded rows / 128
NFR = SEQ * NE          # 1024 frames per core (frame = s*16 + e)
CH = 512                # chunk width
NCH = N // CH           # 32 chunks
NGRP = 8                # groups of 4 chunks (2048 samples)
NBINS = 128             # kept rfft bins

# largest fp32 strictly below 2*pi: mod output [0,1] maps inside [-pi, pi]
SIN_SCALE = float(np.float32(6.28318500518798828125))
SIN_BIAS = -SIN_SCALE / 2.0

_cache = {}


def _interp_u():
    pos = (np.arange(N, dtype=np.float64) + 0.5) * (SEQ / N) - 0.5
    pos = np.clip(pos, 0.0, SEQ - 1)
    i0 = np.floor(pos).astype(np.int64)
    i1 = np.minimum(i0 + 1, SEQ - 1)
    w = pos - i0
    U = np.zeros((SEQ, N))
    U[i0, np.arange(N)] += 1.0 - w
    U[i1, np.arange(N)] += w
    return U


def _build_consts():
    if "consts" in _cache:
        return _cache["consts"]
    U = _interp_u()

    # chunk-local bases from an interior chunk (exact everywhere with the
    # extended-knot convention fx[k] = f[clip(k-1, 0, 63)])
    c0 = 4
    Bp = np.zeros((5, CH))
    Benv = np.zeros((4, CH))
    Bp[0] = 1.0
    for j in range(4):
        u_j = U[2 * c0 - 1 + j, CH * c0: CH * (c0 + 1)]
        Benv[j] = u_j
        Bp[1 + j] = np.cumsum(u_j)
    bh = Bp.astype(np.float16)
    bl = (Bp - bh.astype(np.float64)).astype(np.float16)
    Bp = np.ascontiguousarray(np.concatenate([bh, bh, bl], axis=0))  # (15, 512)
    Benv = np.ascontiguousarray(Benv.astype(np.float16))

    # forward rDFT (t-major), bins 0..127 only; Hann window is pre-applied to
    # the noise frames on the host
    t = np.arange(WS)
    f = np.arange(NBINS)
    ang = 2.0 * np.pi * np.outer(t, f) / WS
    CwRe = np.ascontiguousarray(np.cos(ang).astype(np.float16))
    CwIm = np.ascontiguousarray((-np.sin(ang)).astype(np.float16))

    # inverse rDFT bases (OLA-fused: cols 0:256 -> y[j], cols 256:512 -> y[j+256])
    wgt = np.full(NBINS, 2.0)
    wgt[0] = 1.0
    ang2 = 2.0 * np.pi * np.outer(f, t) / WS
    Dre = np.ascontiguousarray((wgt[:, None] * np.cos(ang2) / WS).astype(np.float16))
    Dim = np.ascontiguousarray((-wgt[:, None] * np.sin(ang2) / WS).astype(np.float16))

    # within-segment interp weight profile for a 256-sample z-slice:
    # j < 128 -> (j+128.5)/256 (right knot weight), j >= 128 -> (j-127.5)/256
    j = np.arange(256)
    w1 = np.where(j < 128, (j + 128.5) / 256.0, (j - 127.5) / 256.0)
    w1bc = np.ascontiguousarray(
        np.broadcast_to(w1.astype(np.float16), (128, 256)).copy())
    w0bc = np.ascontiguousarray(
        np.broadcast_to((1.0 - w1).astype(np.float16), (128, 256)).copy())

    consts = dict(Bp=Bp, Benv=Benv, CwRe=CwRe, CwIm=CwIm,
                  Dre=Dre, Dim=Dim, w1bc=w1bc, w0bc=w0bc)
    _cache["consts"] = consts
    return consts


def _host_prep(inputs):
    """Vectorized host prep over all cores. Returns per-core input maps."""
    f0 = np.asarray(inputs["f0"], np.float32)
    overall_env = np.asarray(inputs["overall_env"], np.float32)
    osc_env = np.asarray(inputs["osc_env"], np.float32)
    harm_env = np.asarray(inputs["harm_env"], np.float32)
    noise_std = np.asarray(inputs["noise_std"], np.float32)
    f0b = np.asarray(inputs["f0_baselines"], np.float32)
    noise_frames = np.asarray(inputs["noise_frames"], np.float32)

    U = _interp_u()

    # match the reference's fp32 frequency values exactly: f0n and f0n*h are
    # rounded to fp32 there, and the phase trajectory follows those values
    f0c = np.clip(f0, -0.5, 0.5)
    erb = ((0.108 * (f0b * NYQ) + 24.7) / NYQ).astype(np.float32)
    f0v = np.clip(f0b + f0c * erb, 0.0, 1.0).astype(np.float32)
    f0n32 = (MIN_F0 + f0v * F0_DIFF).astype(np.float32)          # (B,16,64)
    hfact32 = np.arange(2, 2 + NH, dtype=np.float32)
    harm32 = (f0n32[:, :, None, :] * hfact32[None, None, :, None]).astype(np.float32)
    rows32 = np.concatenate([f0n32[:, :, None, :], harm32], axis=2)  # (B,16,33,64)
    fT = rows32.astype(np.float64).reshape(B, NROW, SEQ) * 0.5   # turns/sample
    f0n = f0n32.astype(np.float64)

    # chunk prefixes P[g, c] = phase before chunk c (fp64 exact), frac-reduced
    V = np.cumsum(U, axis=1)
    Vc = V[:, [CH * c - 1 for c in range(1, NCH)]]               # (64, 31)
    P = np.zeros((B, NROW, NCH))
    P[:, :, 1:] = fT @ Vc

    # extended knots fx[:, :, kk] = f[:, :, clip(kk-1, 0, 63)], kk = 0..65
    ext = np.clip(np.arange(SEQ + 2) - 1, 0, SEQ - 1)

    fx_pad = np.zeros((B, NBLK * 128, SEQ + 2), np.float64)
    fx_pad[:, :NROW] = fT[:, :, ext]
    Pf_pad = np.zeros((B, NBLK * 128, NCH), np.float64)
    Pf_pad[:, :NROW] = np.mod(P, 1.0)

    # phase coefs frc[b][15, NCH*NBLK*128]: hi/lo fp16 split [ch; cl; ch],
    # free index = (c*NBLK + blk)*128 + r
    frc = np.zeros((B, NCH, NBLK, 5, 128), np.float64)
    for c in range(NCH):
        blkv = fx_pad[:, :, 2 * c: 2 * c + 4]                    # (B,640,4)
        kn = blkv.reshape(B, NBLK, 128, 4).transpose(0, 1, 3, 2)  # (B,5,4,128)
        frc[:, c, :, 1:5, :] = kn
        frc[:, c, :, 0, :] = Pf_pad[:, :, c].reshape(B, NBLK, 128)
    ch = frc.astype(np.float16)
    cl = (frc - ch.astype(np.float64)).astype(np.float16)
    frc = np.concatenate([ch, cl, ch], axis=3)                   # (B,NCH,NBLK,15,128)
    frc = np.ascontiguousarray(
        frc.transpose(0, 3, 1, 2, 4).reshape(B, 15, NCH * NBLK * 128))

    # env-valued (negated) selection weights per extended knot kk and block b:
    # layout [128, 16 + (kk*NBLK+b)*32 ...]: 16 leading zeros, then per index
    # i = kk*NBLK+b a [16 w-cols | 16 zero-cols] pair. The u=0 stationary
    # slice is cols [16+32i, 16+32i+32) = [w | 0]; u=1 is [32i, 32i+32) = [0 | w].
    oe = np.clip(osc_env, 0.0, 1.0).astype(np.float32)
    he = np.clip(harm_env, 0.0, 1.0).astype(np.float32)
    env_rows = oe[:, :, None, :] * np.concatenate(
        [np.ones((B, NE, 1, SEQ), np.float32), he], axis=2)
    env_rows = env_rows.reshape(B, NROW, SEQ)
    NKK = SEQ + 2
    evw = np.zeros((B, NKK, NBLK, 128, 16), np.float16)
    gidx = np.arange(NROW)
    ev = gidx // 33
    blk = gidx // 128
    rr = gidx % 128
    envx = -env_rows[:, :, ext]                                  # (B,528,66)
    evw[:, :, blk, rr, ev] = envx[:, gidx].transpose(0, 2, 1)
    ewpad = np.zeros((B, NKK, NBLK, 128, 32), np.float16)
    ewpad[..., 0:16] = evw
    ew = np.zeros((B, 128, 16 + NKK * NBLK * 32), np.float16)
    ew[:, :, 16:] = ewpad.transpose(0, 3, 1, 2, 4).reshape(B, 128, NKK * NBLK * 32)
    ew = np.ascontiguousarray(ew)

    # overall mix coefs, u-split: ovc[b][4, (c*2+u)*32 + row], zero outside
    # the u-th 16-row group
    ov = np.clip(overall_env, 0.0, 1.0).astype(np.float32)
    ovx = ov[:, :, ext]                                          # (B,16,66)
    ovc = np.zeros((B, NCH, 2, 4, 32), np.float16)
    for c in range(NCH):
        k = ovx[:, :, 2 * c: 2 * c + 4].transpose(0, 2, 1)       # (B,4,16)
        ovc[:, c, 0, :, 0:16] = k
        ovc[:, c, 1, :, 16:32] = k
    ovc = np.ascontiguousarray(
        ovc.transpose(0, 3, 1, 2, 4).reshape(B, 4, NCH * 64))

    # noise: mean/c2 rows frame-major (fr = s*16 + e)
    std = (np.clip(noise_std, 1e-12, 1.0) * F0_DIFF).astype(np.float32)
    c2 = (-0.5 / (std.astype(np.float64) ** 2)).astype(np.float32)
    f0n32 = f0n.astype(np.float32)
    mrow = f0n32.transpose(0, 2, 1).reshape(B, 1, NFR).astype(np.float64)
    crow = c2.transpose(0, 2, 1).reshape(B, 1, NFR).astype(np.float64)
    freq = (np.arange(NBINS) / 256.0).reshape(1, NBINS, 1)
    filt = np.exp(crow * (freq - mrow) ** 2)                     # (B,128,NFR)
    filt = np.ascontiguousarray(filt.astype(np.float16))

    # pre-windowed, transposed noise frames: nfT[b][t, fr] fp16
    t = np.arange(WS)
    win = (0.5 - 0.5 * np.cos(2.0 * np.pi * t / WS)).astype(np.float32)
    nf = noise_frames.transpose(0, 2, 1, 3).reshape(B, NFR, WS)  # (B, fr, t)
    nfT = np.ascontiguousarray(
        (nf * win[None, None, :]).transpose(0, 2, 1).astype(np.float16))

    consts = _build_consts()
    in_maps = []
    for b in range(B):
        m = dict(frc=frc[b], ew=ew[b], ovc=ovc[b], filt=filt[b], nfT=nfT[b])
        m.update(consts)
        in_maps.append(m)
    return in_maps


def _build_nc():
    if "nc" in _cache:
        return _cache["nc"]
    from concourse import bacc
    import concourse.tile as tile
    from concourse import mybir
    from contextlib import ExitStack

    F32 = mybir.dt.float32
    F32R = mybir.dt.float32r
    FP16 = mybir.dt.float16
    AF = mybir.ActivationFunctionType
    OP = mybir.AluOpType

    nc = bacc.Bacc()
    frc = nc.declare_dram_parameter("frc", [15, NCH * NBLK * 128], FP16, isOutput=False)
    ew = nc.declare_dram_parameter("ew", [128, 16 + (SEQ + 2) * NBLK * 32], FP16, isOutput=False)
    ovc = nc.declare_dram_parameter("ovc", [4, NCH * 64], FP16, isOutput=False)
    filt = nc.declare_dram_parameter("filt", [NBINS, NFR], FP16, isOutput=False)
    nfT = nc.declare_dram_parameter("nfT", [WS, NFR], FP16, isOutput=False)
    Bp = nc.declare_dram_parameter("Bp", [15, CH], FP16, isOutput=False)
    Benv = nc.declare_dram_parameter("Benv", [4, CH], FP16, isOutput=False)
    CwRe = nc.declare_dram_parameter("CwRe", [WS, NBINS], FP16, isOutput=False)
    CwIm = nc.declare_dram_parameter("CwIm", [WS, NBINS], FP16, isOutput=False)
    Dre = nc.declare_dram_parameter("Dre", [NBINS, WS], FP16, isOutput=False)
    Dim = nc.declare_dram_parameter("Dim", [NBINS, WS], FP16, isOutput=False)
    w1bc = nc.declare_dram_parameter("w1bc", [128, 256], FP16, isOutput=False)
    w0bc = nc.declare_dram_parameter("w0bc", [128, 256], FP16, isOutput=False)
    out = nc.declare_dram_parameter("out", [NE, N], F32, isOutput=True)

    CB = 2 * NBLK * 128   # coef columns per c2 iteration (1280)

    with tile.TileContext(nc) as tc, ExitStack() as ctx:
        cp = ctx.enter_context(tc.tile_pool(name="const", bufs=1))

        # noise inputs first so the noise pipeline starts ASAP
        nft_sb = [cp.tile([128, NFR], FP16, tag=f"nft{t4}", name=f"nft{t4}")
                  for t4 in range(4)]
        for t4 in range(4):
            nc.sync.dma_start(nft_sb[t4][:], nfT[t4 * 128:(t4 + 1) * 128, :])
        cwre_sb = cp.tile([128, 4 * NBINS], FP16, tag="cwre")
        cwim_sb = cp.tile([128, 4 * NBINS], FP16, tag="cwim")
        for t4 in range(4):
            nc.sync.dma_start(cwre_sb[:, t4 * NBINS:(t4 + 1) * NBINS],
                              CwRe[t4 * 128:(t4 + 1) * 128, :])
            nc.sync.dma_start(cwim_sb[:, t4 * NBINS:(t4 + 1) * NBINS],
                              CwIm[t4 * 128:(t4 + 1) * 128, :])
        ff_sb = cp.tile([NBINS, NFR], FP16, tag="ff")
        nc.sync.dma_start(ff_sb[:], filt[:])
        dre_sb = cp.tile([NBINS, WS], FP16, tag="dre")
        nc.sync.dma_start(dre_sb[:], Dre[:])
        dim_sb = cp.tile([NBINS, WS], FP16, tag="dim")
        nc.sync.dma_start(dim_sb[:], Dim[:])

        # osc constants
        bp_sb = cp.tile([15, CH], FP16, tag="bp")
        nc.sync.dma_start(bp_sb[:], Bp[:])
        benv_sb = cp.tile([4, CH], FP16, tag="benv")
        nc.sync.dma_start(benv_sb[:], Benv[:])
        w1_sb = cp.tile([128, 256], FP16, tag="w1bc")
        nc.sync.dma_start(w1_sb[:], w1bc[:])
        w0_sb = cp.tile([128, 256], FP16, tag="w0bc")
        nc.sync.dma_start(w0_sb[:], w0bc[:])
        ew_sb = cp.tile([128, 16 + (SEQ + 2) * NBLK * 32], FP16, tag="ew")
        ovc_sb = cp.tile([4, NCH * 64], FP16, tag="ovc")
        nc.sync.dma_start(ovc_sb[:], ovc[:])
        bsin = cp.tile([128, 1], F32, tag="bsin")
        nc.vector.memset(bsin[:], 0.0)
        b1024 = cp.tile([128, 1], F32, tag="b1024")
        nc.vector.memset(b1024[:], 1024.0)

        # ---------------- noise phase (PSUM pools scoped) ----------------
        na = ctx.enter_context(tc.tile_pool(name="na", bufs=1))
        nzS = [na.tile([128, 256], FP16, tag=f"nz{g}", name=f"nz{g}")
               for g in range(NGRP)]
        with tc.tile_pool(name="psN", bufs=1, space="PSUM") as psN, \
             tc.tile_pool(name="psZ", bufs=2, space="PSUM") as psZ:
            # rfft (bins 0..127) + filter; specf tiles have 16 zero lead cols
            sfre = na.tile([128, 16 + NFR], FP16, tag="sfre")
            sfim = na.tile([128, 16 + NFR], FP16, tag="sfim")
            nc.vector.memset(sfre[:, 0:16], 0.0)
            nc.vector.memset(sfim[:, 0:16], 0.0)
            for h in range(2):
                sl = slice(h * 512, (h + 1) * 512)
                spr = psN.tile([128, 512], F32, tag="spr")
                spi = psN.tile([128, 512], F32, tag="spi")
                for t4 in range(4):
                    nc.tensor.matmul(spr[:], cwre_sb[:, t4 * NBINS:(t4 + 1) * NBINS],
                                     nft_sb[t4][:, sl], start=(t4 == 0), stop=(t4 == 3))
                for t4 in range(4):
                    nc.tensor.matmul(spi[:], cwim_sb[:, t4 * NBINS:(t4 + 1) * NBINS],
                                     nft_sb[t4][:, sl], start=(t4 == 0), stop=(t4 == 3))
                nc.vector.tensor_tensor(sfre[:, 16 + h * 512:16 + (h + 1) * 512],
                                        spr[:], ff_sb[:, sl], OP.mult)
                nc.vector.tensor_tensor(sfim[:, 16 + h * 512:16 + (h + 1) * 512],
                                        spi[:], ff_sb[:, sl], OP.mult)

            # inverse rDFT + OLA fused; nzS[g] in z-major [16z+e, 256j] fp16
            for g in range(NGRP):
                nzp = psZ.tile([128, 256], F32, tag="nzp")
                nc.tensor.matmul(nzp[:], sfre[:, 16 + g * 128: 16 + g * 128 + 128],
                                 dre_sb[:, 0:256], start=True, stop=False)
                nc.tensor.matmul(nzp[:], sfim[:, 16 + g * 128: 16 + g * 128 + 128],
                                 dim_sb[:, 0:256], start=False, stop=False)
                nc.tensor.matmul(nzp[:], sfre[:, g * 128: g * 128 + 128],
                                 dre_sb[:, 256:512], start=False, stop=False)
                nc.tensor.matmul(nzp[:], sfim[:, g * 128: g * 128 + 128],
                                 dim_sb[:, 256:512], start=False, stop=True)
                nc.scalar.copy(nzS[g][:], nzp[:])

        # ---------------- oscillator phase ----------------
        cof = ctx.enter_context(tc.tile_pool(name="cof", bufs=4))
        dsp = ctx.enter_context(tc.tile_pool(name="dsp", bufs=8))
        stp = ctx.enter_context(tc.tile_pool(name="stp", bufs=12))
        ocp = ctx.enter_context(tc.tile_pool(name="ocp", bufs=2))
        psA = ctx.enter_context(tc.tile_pool(name="psA", bufs=2, space="PSUM"))
        psA2 = ctx.enter_context(tc.tile_pool(name="psA2", bufs=1, space="PSUM"))
        psG = ctx.enter_context(tc.tile_pool(name="psG", bufs=1, space="PSUM"))
        psM = ctx.enter_context(tc.tile_pool(name="psM", bufs=1, space="PSUM"))

        def ew_sl(kk, b, u):
            i = kk * NBLK + b
            base = 16 + 32 * i if u == 0 else 32 * i
            return ew_sb[:, base: base + 32]

        gbans = {}
        pmbans = {}

        EWC = 16 + (SEQ + 2) * NBLK * 32

        def emit_front(c2):
            """Phase matmuls + range reduction + sin for iteration c2."""
            fc_sb = cof.tile([15, CB], FP16, tag="fc")
            nc.sync.dma_start(fc_sb[:], frc[:, c2 * CB:(c2 + 1) * CB])
            if c2 < 4:
                # stream the env-weight table in quarters so its large DMA
                # never blocks the early coefficient DMAs
                q0 = (EWC // 4) * c2
                q1 = EWC if c2 == 3 else (EWC // 4) * (c2 + 1)
                nc.sync.dma_start(ew_sb[:, q0:q1], ew[:, q0:q1])
            # stage-ordered emission: each engine's queue is sorted so no
            # op head-blocks behind one waiting on a slower producer.
            # range reduction: yt = fp16(phase + 1024) rounds to the nearest
            # integer (fp16 ulp is exactly 1 in [1024, 2048)); then
            # -dt = (yt - 1024) - phase via scalar_tensor_tensor.
            order = (3, 4, 0, 1, 2)
            pas = {}
            yts = {}
            dts = {}
            sts = [None] * NBLK
            for b in order:
                pool = psA2 if b == 0 else psA
                pa = pool.tile([128, 1024], F32, tag="pa")
                for ci in range(2):
                    idx = (ci * NBLK + b) * 128
                    nc.tensor.matmul(pa[:, ci * 512:(ci + 1) * 512],
                                     fc_sb[:, idx:idx + 128],
                                     bp_sb[:], start=True, stop=True)
                pas[b] = pa
            for b in order:
                yt = dsp.tile([128, 1024], FP16, tag="yt")
                if b < 3:
                    nc.scalar.activation(yt[:], pas[b][:], AF.Identity,
                                         bias=b1024[:], scale=1.0)
                else:
                    nc.vector.tensor_scalar(yt[:], pas[b][:], 1024.0, None, OP.add)
                yts[b] = yt
            for b in order:
                dt_ = dsp.tile([128, 1024], FP16, tag="dt")
                nc.vector.scalar_tensor_tensor(dt_[:], yts[b][:], 1024.0,
                                               pas[b][:], OP.subtract, OP.subtract)
                dts[b] = dt_
            for b in order:
                st = stp.tile([128, 1024], FP16, tag="st")
                nc.scalar.activation(st[:], dts[b][:], AF.Sin, bias=bsin[:],
                                     scale=SIN_SCALE)
                sts[b] = (st, 0)
            return sts

        def emit_back(c2, sts):
            cA = 2 * c2
            grp = cA // 4
            if cA % 4 == 0:
                gbans[grp] = psG.tile([128, 512], F32, tag="gb", name=f"gb{grp}")
                pmbans[grp] = psM.tile([128, 256], F32, tag="pm", name=f"pm{grp}")
            gb = gbans[grp]
            pm = pmbans[grp]

            # env-weighted selection matmuls: Glo (gb cols 0:256) holds the
            # left-knot weighted harmonic sum, Ghi (cols 256:512) the right
            for ci in range(2):
                cc = cA + ci
                zp = cc % 4
                for u in range(2):
                    mkk = 2 * cc + u
                    for b in range(NBLK):
                        stt_, base = sts[b]
                        o0 = base + ci * 512 + u * 256
                        stA = stt_[:, o0: o0 + 128]
                        stB = stt_[:, o0 + 128: o0 + 256]
                        # start=True only on the very first matmul touching this
                        # 32-row bank region: its start marks the whole zero
                        # region pending, and first-touch zeroing initializes
                        # the other column ranges
                        fb_ = (u == 0 and b == 0)
                        lb_ = (u == 1 and b == NBLK - 1)
                        nc.tensor.matmul(gb[32 * zp:32 * zp + 32, 0:128],
                                         ew_sl(mkk, b, u), stA,
                                         start=fb_, stop=False,
                                         skip_group_check=True, tile_position=(0, 32 * zp))
                        nc.tensor.matmul(gb[32 * zp:32 * zp + 32, 128:256],
                                         ew_sl(mkk + 1, b, u), stB,
                                         start=False, stop=False,
                                         skip_group_check=True, tile_position=(0, 32 * zp))
                        nc.tensor.matmul(gb[32 * zp:32 * zp + 32, 256:384],
                                         ew_sl(mkk + 1, b, u), stA,
                                         start=False, stop=False,
                                         skip_group_check=True, tile_position=(0, 32 * zp))
                        nc.tensor.matmul(gb[32 * zp:32 * zp + 32, 384:512],
                                         ew_sl(mkk + 2, b, u), stB,
                                         start=False, stop=lb_,
                                         skip_group_check=True, tile_position=(0, 32 * zp))
                # mix interp into pm (z-major rows)
                for u in range(2):
                    nc.tensor.matmul(
                        pm[32 * zp:32 * zp + 32, 0:256],
                        ovc_sb[:, (cc * 2 + u) * 32:(cc * 2 + u + 1) * 32],
                        benv_sb[:, u * 256:(u + 1) * 256],
                        start=(u == 0), stop=(u == 1), skip_group_check=True,
                        tile_position=(0, 32 * zp))

            if cA % 4 == 2:
                # blend: posc = w0*Glo + w1*Ghi (each op reads <=1 PSUM input)
                glo = gb[:, 0:256]
                ghi = gb[:, 256:512]
                t1 = ocp.tile([128, 256], FP16, tag="t1")
                nc.vector.tensor_tensor(t1[:], glo, w0_sb[:], OP.mult)
                t2 = ocp.tile([128, 256], FP16, tag="t2")
                nc.vector.tensor_tensor(t2[:], ghi, w1_sb[:], OP.mult)
                posc_s = ocp.tile([128, 256], F32, tag="ps")
                nc.gpsimd.tensor_tensor(posc_s[:], t1[:], t2[:], OP.add)
                pc = ocp.tile([128, 256], FP16, tag="pc")
                nc.scalar.copy(pc[:], pm[:])
                av = ocp.tile([128, 256], F32, tag="av")
                nc.gpsimd.tensor_tensor(av[:], posc_s[:], nzS[grp][:], OP.subtract)
                bv = ocp.tile([128, 256], F32, tag="bv")
                nc.gpsimd.tensor_tensor(bv[:], av[:], pc[:], OP.mult)
                ov_ = ocp.tile([128, 256], F32, tag="ov")
                nc.gpsimd.tensor_tensor(ov_[:], bv[:], nzS[grp][:], OP.add)
                for z in range(8):
                    nc.sync.dma_start(
                        out[:, grp * 2048 + z * 256: grp * 2048 + (z + 1) * 256],
                        ov_[16 * z:16 * (z + 1), :])
                del gbans[grp]
                del pmbans[grp]

        # software-pipelined emission: the back half (selection matmuls and
        # combine) lags one iteration behind the front half (phase/sin), so
        # the PE never stalls waiting on freshly produced sins
        pending = None
        for c2 in range(NCH // 2):
            sts_new = emit_front(c2)
            if pending is not None:
                emit_back(*pending)
            pending = (c2, sts_new)
        emit_back(*pending)

    nc.finalize()
    _cache["nc"] = nc
    return nc


def kernel(**inputs):
    from concourse.bass_utils import run_bass_kernel_spmd

    in_maps = _host_prep(inputs)
    nc = _build_nc()

    trace = bool(os.environ.get("BASS_PROFILE"))
    res = run_bass_kernel_spmd(nc, in_maps, list(range(B)), trace=trace)
    if trace and res.exec_time_ns is not None:
        print(f"HW exec time: {res.exec_time_ns} ns")
    out = np.stack([r["out"] for r in res.results]).astype(np.float32)
    return out
